# revision 1
# baseline (speedup 1.0000x reference)
"""Bidirectional RNN (embed -> fwd/bwd tanh scans -> vocab projection) on 8
TRN2 NeuronCores.

Strategy (per core, SPMD, identical program, no collectives):
  - Direction-split data parallelism: cores 0-3 run the FORWARD scan, cores
    4-7 the BACKWARD scan. The direction is chosen purely by the per-core
    input data (reversed token order + that direction's weights), so the
    instruction stream is identical on all cores.
  - The fc projection is split along BOTH vocab (4 column slices of 8000)
    and the contraction (each core uses only its own direction's 512 rows of
    W_fc). Core c and core c+4 produce additive partials for the same vocab
    slice; the host sums them (plus b_fc). This halves per-core scan work
    and removes any cross-direction dependency, so fc matmuls overlap the
    scan's latency bubbles.
  - Embedding gather is an indirect DMA per 128 tokens; gathered rows are
    PE-transposed and projected (x @ W_xh + b_h) chunk-by-chunk, with the
    scan consuming each chunk's staging tile directly from SBUF.

Layouts:
  - packed hidden-state step block: column m*16 + b for hidden row m*128+p
    (m: 4 H-tiles, b: batch 16) -> one PSUM bank [128, 64].
  - h ring [128, 4*513*16]: column m*8208 + slot*16 + b. Slot s+1 holds the
    state after scan step s (scan order). fc reads 128-column contiguous
    runs as matmul stationary operands. Backward cores' output rows come
    out time-reversed; the host flips them.
"""
import numpy as np

import concourse.bacc as bacc
import concourse.bass as bass
import concourse.mybir as mybir
import concourse.tile as tile
from concourse.bass_utils import run_bass_kernel_spmd
from concourse.masks import make_identity

P = 128
VOCAB, EMBED, HIDDEN = 32000, 256, 512
B, T = 16, 512
NCORES = 8
VSLICE = VOCAB // 4               # 8000 vocab cols per core (pairs share)
PANW = 500                        # cols per PSUM chunk
NCHUNK_V = VSLICE // PANW         # 16
MT = HIDDEN // P                  # 4 hidden tiles
ET = EMBED // P                   # 2 embed tiles
NTOK = B * T                      # 8192
NG = NTOK // P                    # 64 gathers
CHTOK = 512                       # tokens per prologue chunk
NCH = NTOK // CHTOK               # 16 chunks
SPC = CHTOK // B                  # 32 steps per chunk
SLOT = 16
MBLK = (T + 1) * SLOT             # 8208 h-ring cols per m-block
BF = mybir.dt.bfloat16
F32 = mybir.dt.float32

_CACHED_NC = None


def build():
    nc = bacc.Bacc(None, target_bir_lowering=False, debug=False)

    emb = nc.declare_dram_parameter("emb", [VOCAB, EMBED], F32, isOutput=False)
    ids_in = nc.declare_dram_parameter("ids_a", [P, NG], mybir.dt.int32,
                                       isOutput=False)
    whh_in = nc.declare_dram_parameter("whh_a", [HIDDEN, HIDDEN], F32,
                                       isOutput=False)
    wxh_in = nc.declare_dram_parameter("wxh_a", [EMBED, HIDDEN], F32,
                                       isOutput=False)
    bh_in = nc.declare_dram_parameter("bh_a", [1, HIDDEN], F32, isOutput=False)
    h0_in = nc.declare_dram_parameter("h0", [P, MT * B], F32, isOutput=False)
    wfc_in = nc.declare_dram_parameter("wfc_a", [HIDDEN, VSLICE], F32,
                                       isOutput=False)
    out = nc.declare_dram_parameter("out", [NTOK, VSLICE], F32, isOutput=True)

    from contextlib import ExitStack
    with tile.TileContext(nc) as tc:
        with tc.tile_pool(name="const", bufs=1) as const, \
             tc.tile_pool(name="hpool", bufs=1) as hpool, \
             tc.tile_pool(name="wfcp", bufs=1) as wfcp, \
             tc.tile_pool(name="evp", bufs=1) as evp, \
             tc.tile_pool(name="ps", bufs=2, space="PSUM") as ps:
            stackA = ExitStack()
            stage = stackA.enter_context(tc.tile_pool(name="stage", bufs=2))
            gat = stackA.enter_context(tc.tile_pool(name="gat", bufs=2))
            xtp = stackA.enter_context(tc.tile_pool(name="xt", bufs=2))
            prest = stackA.enter_context(tc.tile_pool(name="prest", bufs=3))

            # ---------------- constants ----------------
            ident_f = const.tile([P, P], F32, tag="ident_f")
            make_identity(nc, ident_f[:])
            ident_b = const.tile([P, P], BF, tag="ident_b")
            nc.vector.tensor_copy(out=ident_b[:], in_=ident_f[:])
            ones_row = const.tile([1, CHTOK], BF, tag="ones_row")
            nc.gpsimd.memset(ones_row[:], 1.0)

            whh = {}
            for kt in range(MT):
                wf = stage.tile([P, HIDDEN], F32, tag="wstage", name="wf")
                nc.sync.dma_start(out=wf[:], in_=whh_in[kt * P:(kt + 1) * P, :])
                for mt in range(MT):
                    wc = const.tile([P, P], BF, tag=f"whh{kt}{mt}", name="wc")
                    nc.vector.tensor_copy(out=wc[:],
                                          in_=wf[:, mt * P:(mt + 1) * P])
                    whh[(kt, mt)] = wc
            wxh = {}
            for e in range(ET):
                wf2 = stage.tile([P, HIDDEN], F32, tag="wstage", name="wf2")
                nc.sync.dma_start(out=wf2[:], in_=wxh_in[e * P:(e + 1) * P, :])
                for mt in range(MT):
                    wc2 = const.tile([P, P], BF, tag=f"wxh{e}{mt}", name="wc2")
                    nc.vector.tensor_copy(out=wc2[:],
                                          in_=wf2[:, mt * P:(mt + 1) * P])
                    wxh[(e, mt)] = wc2
            btf = stage.tile([1, HIDDEN], F32, tag="bstage", name="btf")
            nc.sync.dma_start(out=btf[:], in_=bh_in[:, :])
            bh = const.tile([1, HIDDEN], BF, tag="bh", name="bh")
            nc.vector.tensor_copy(out=bh[:], in_=btf[:])
            ids_sb = const.tile([P, NG], mybir.dt.int32, tag="ids", name="ids")
            nc.sync.dma_start(out=ids_sb[:], in_=ids_in[:, :])
            h0f = const.tile([P, MT * B], F32, tag="h0f")
            nc.sync.dma_start(out=h0f[:], in_=h0_in[:, :])

            # W_fc resident: 4 k-tiles [128, VSLICE] bf16, converted in
            # column chunks through a small f32 staging tile
            wfc = {}
            for kt in range(MT):
                wfb = wfcp.tile([P, VSLICE], BF, tag=f"wfc{kt}", name="wfb")
                wfc[kt] = wfb
            for kt in range(MT):
                for q in range(4):
                    qw = VSLICE // 4
                    wfs = stage.tile([P, qw], F32, tag="wfcstage", name="wfs")
                    nc.sync.dma_start(
                        out=wfs[:],
                        in_=wfc_in[kt * P:(kt + 1) * P, q * qw:(q + 1) * qw])
                    nc.vector.tensor_copy(out=wfc[kt][:, q * qw:(q + 1) * qw],
                                          in_=wfs[:])

            # h ring
            hbig = hpool.tile([P, MT * MBLK], BF, tag="hbig", name="hbig")

            def hslot_w(slot):
                return hbig[:].rearrange(
                    "p (m s) -> p m s", m=MT)[:, :, slot * SLOT:(slot + 1) * SLOT]

            def hslot_r(slot, kt):
                base = kt * MBLK + slot * SLOT
                return hbig[:, base:base + SLOT]

            nc.vector.tensor_copy(
                out=hslot_w(0), in_=h0f[:].rearrange("p (m s) -> p m s", m=MT))

            evict_flip = [0]

            def evict_engine():
                evict_flip[0] ^= 1
                return nc.vector if evict_flip[0] else nc.scalar

            # ---------------- chunk prologue ----------------
            stg_cur = [None]
            xg_pend = {}

            def emit_gathers(c):
                for g in range(CHTOK // P):
                    gi = c * (CHTOK // P) + g
                    xg = gat.tile([P, EMBED], F32, tag=f"xg{g}", name="xg")
                    nc.gpsimd.indirect_dma_start(
                        out=xg[:], out_offset=None, in_=emb[:],
                        in_offset=bass.IndirectOffsetOnAxis(
                            ap=ids_sb[:, gi:gi + 1], axis=0),
                    )
                    xg_pend[(c, g)] = xg

            def emit_chunk(c):
                xt = {e: xtp.tile([P, CHTOK], BF, tag=f"xt{e}", name=f"xt{e}")
                      for e in range(ET)}
                for g in range(CHTOK // P):
                    xg = xg_pend.pop((c, g))
                    for e in range(ET):
                        tp = ps.tile([P, P], F32, tag="big0", name="tp")
                        nc.tensor.transpose(
                            out=tp[:], in_=xg[:, e * P:(e + 1) * P],
                            identity=ident_f[:])
                        nc.vector.tensor_copy(
                            out=xt[e][:, g * P:(g + 1) * P], in_=tp[:])
                stg = prest.tile([P, SPC * MT * B], BF, tag="prestg",
                                 name="stg")
                stg3 = stg[:].rearrange("p (s m) -> p s m", m=MT * B)
                for mt in range(MT):
                    zp = ps.tile([P, CHTOK], F32, tag=f"big{mt % 2}",
                                 name="zp")
                    for e in range(ET):
                        nc.tensor.matmul(
                            out=zp[:], lhsT=wxh[(e, mt)][:], rhs=xt[e][:],
                            start=(e == 0), stop=False, skip_group_check=True)
                    nc.tensor.matmul(
                        out=zp[:], lhsT=bh[:, mt * P:(mt + 1) * P],
                        rhs=ones_row[:], start=False, stop=True,
                        skip_group_check=True)
                    dst = stg3[:, :, mt * B:(mt + 1) * B]
                    eng = evict_engine()
                    if eng is nc.scalar:
                        nc.scalar.activation(
                            out=dst, in_=zp[:],
                            func=mybir.ActivationFunctionType.Copy)
                    else:
                        nc.vector.tensor_copy(out=dst, in_=zp[:])
                stg_cur[0] = stg

            # ---------------- fc for one token M-tile ----------------
            def emit_fc_mtile(mt):
                t0 = mt * 8
                for vch in range(NCHUNK_V):
                    z = ps.tile([P, PANW], F32, tag=f"big{vch % 2}", name="z")
                    for kt in range(MT):
                        lhsT = hbig[:, kt * MBLK + (t0 + 1) * SLOT:
                                    kt * MBLK + (t0 + 1) * SLOT + P]
                        nc.tensor.matmul(out=z[:], lhsT=lhsT,
                                         rhs=wfc[kt][:, vch * PANW:
                                                     (vch + 1) * PANW],
                                         start=(kt == 0), stop=(kt == MT - 1))
                    ev = evp.tile([P, PANW], F32, tag=f"ev{vch % 4}", name="ev")
                    eng = evict_engine()
                    if eng is nc.scalar:
                        nc.scalar.activation(
                            out=ev[:], in_=z[:],
                            func=mybir.ActivationFunctionType.Copy)
                    else:
                        nc.vector.tensor_copy(out=ev[:], in_=z[:])
                    nc.sync.dma_start(
                        out=out[mt * P:(mt + 1) * P,
                                vch * PANW:(vch + 1) * PANW],
                        in_=ev[:])

            # ---------------- main loop: chunk -> 32 steps -> 4 fc tiles ----
            emit_gathers(0)
            for c in range(NCH):
                if c + 1 < NCH:
                    emit_gathers(c + 1)
                emit_chunk(c)
                for s in range(c * SPC, (c + 1) * SPC):
                    z = ps.tile([P, MT * B], F32, tag=f"zscan{s % 2}",
                                name="z")
                    nc.tensor.matmul(
                        out=z[:], lhsT=ident_b[:],
                        rhs=stg_cur[0][:, (s % SPC) * 64:(s % SPC) * 64 + 64],
                        start=True, stop=False, skip_group_check=True)
                    for mt in range(MT):
                        for kt in range(MT):
                            nc.tensor.matmul(
                                out=z[:, mt * B:(mt + 1) * B],
                                lhsT=whh[(kt, mt)][:],
                                rhs=hslot_r(s, kt),
                                start=False,
                                stop=(mt == MT - 1 and kt == MT - 1),
                                skip_group_check=True)
                    nc.scalar.activation(
                        out=hslot_w(s + 1), in_=z[:],
                        func=mybir.ActivationFunctionType.Tanh)
                for mt in range(4 * c, 4 * c + 4):
                    emit_fc_mtile(mt)

            stackA.close()
    nc.finalize()
    return nc


def _pack_h(hT):
    # [H, B] -> [128, MT*B] packed (col = m*16+b)
    return np.ascontiguousarray(
        hT.reshape(MT, P, B).transpose(1, 0, 2).reshape(P, MT * B))


def make_in_maps(inputs, h_prev, emb, W_xh_f, W_hh_f, b_h_f,
                 W_xh_b, W_hh_b, b_h_b, W_fc, b_fc):
    inputs = np.asarray(inputs, dtype=np.int32)
    ids = {"f": inputs, "b": inputs[:, ::-1]}
    W_xh = {"f": np.asarray(W_xh_f, np.float32),
            "b": np.asarray(W_xh_b, np.float32)}
    W_hh = {"f": np.asarray(W_hh_f, np.float32),
            "b": np.asarray(W_hh_b, np.float32)}
    b_h = {"f": np.asarray(b_h_f, np.float32),
           "b": np.asarray(b_h_b, np.float32)}
    W_fc = np.asarray(W_fc, np.float32)
    emb = np.ascontiguousarray(np.asarray(emb, dtype=np.float32))
    h0 = _pack_h(np.asarray(h_prev, np.float32).T)

    in_maps = []
    for c in range(NCORES):
        d = "f" if c < 4 else "b"
        j = c % 4
        krows = slice(0, HIDDEN) if d == "f" else slice(HIDDEN, 2 * HIDDEN)
        m = {
            "emb": emb,
            "ids_a": np.ascontiguousarray(ids[d].T.reshape(NG, P).T),
            "whh_a": W_hh[d],
            "wxh_a": W_xh[d],
            "bh_a": np.ascontiguousarray(b_h[d].reshape(1, HIDDEN)),
            "h0": h0,
            "wfc_a": np.ascontiguousarray(
                W_fc[krows, j * VSLICE:(j + 1) * VSLICE]),
        }
        in_maps.append(m)
    return in_maps


def assemble(results, b_fc):
    # core j (fwd) + core j+4 (bwd, time-reversed rows) sum to a vocab slice
    cols = []
    for j in range(4):
        f = results[j]["out"]
        bk = results[j + 4]["out"].reshape(T, B, VSLICE)[::-1].reshape(
            NTOK, VSLICE)
        cols.append(f + bk)
    full = np.concatenate(cols, axis=1)          # [8192, 32000], (t, b) rows
    full = full.reshape(T, B, VOCAB).transpose(1, 0, 2)
    return np.ascontiguousarray(full + np.asarray(b_fc, np.float32))


def kernel(inputs, h_prev, emb, W_xh_f, W_hh_f, b_h_f,
           W_xh_b, W_hh_b, b_h_b, W_fc, b_fc):
    global _CACHED_NC
    if _CACHED_NC is None:
        _CACHED_NC = build()
    in_maps = make_in_maps(inputs, h_prev, emb, W_xh_f, W_hh_f, b_h_f,
                           W_xh_b, W_hh_b, b_h_b, W_fc, b_fc)
    res = run_bass_kernel_spmd(_CACHED_NC, in_maps,
                               core_ids=list(range(NCORES)))
    return assemble(res.results, b_fc)

